# revision 33
# baseline (speedup 1.0000x reference)
"""Discriminative-loss (clustering) kernel for Trainium2, 8 NeuronCores.

Data parallel over batch (B=16 -> 2 images/core). Per image the device
computes, for labels l=0..4:
    sums[l, e]  = sum_p [inst==l]*bin*x_e      (masked, channels 0..7)
    sumsq[l, e] = sum_p [inst==l]*bin*x_e^2    (channels 8..15, host sums e)
    counts[l]   = sum_p [inst==l]              (raw, ones channel 16)
The tiny remaining math (means, hinge, pairwise distances -> scalar) runs
on the host in float64.

Device design (cost-model-driven):
  - DMA casts shrink charged HBM traffic to ~10MB/core: pred f32->fp8e4
    (8MB), inst int32->int8 (1MB), binl(*2 on host) f32->fp8 (1MB). All
    casts ride SWDGE (gpsimd).
  - Masks are built BIT-WISE on DVE in packed int16 (2 pixels per lane,
    4x DVE mode): labels 0..4 are small ints, so [it==l] per byte is
      t = itp ^ l*0x0101; t += 0x3F3F; raw = (t & 0x4040) ^ 0x4040
    giving fp8 bit pattern 0x40 (=2.0) at matches. Masked planes are
    raw & binp (binp = fp8 image of 2*binary = 0x40). walrus rejects
    fused bitwise+arith tensor_scalar, so: 5 per-label XORs + merged
    [P,5,W] add/extract/AND. All ops chunked so matmuls start early.
  - Squares (slots 8..15, fp8) split across ACT / DVE / Pool by column
    ranges - the dominant elementwise cost (65536 cols/core).
  - TensorE: fp8 DoubleRow matmuls (0.5 cyc/row, k=256 pixels): one
    matmul per pixel-column pair with lhsT = mask planes [p, 2, 10] and
    rhs = data slots [p, 2, 17]; psum [10, 17] accumulates a full
    image. The dual-fp8 ldweights ISA check rejects byte-interleaved
    k-pairs, so pairs are (c, c+128) within 256-col groups.
"""

import numpy as np

import concourse.mybir as mybir
from concourse import bacc, bass_utils
from concourse.tile import TileContext

P = 128
FH = 2048          # pixel cols per half-image tile
WH = FH // 2       # int16 words per half tile
NLAB = 5
NPL = 10           # mask planes: 0..4 masked, 5..9 raw
NCH = 17           # rhs slots: 0-7 pred, 8-15 pred^2, 16 ones
GRP = 256          # pixel cols per matmul group (k-pair stride = GRP//2)
BPC = 2            # images per core
NCORES = 8
SB = 1024          # sub-block: unit of mask/square/matmul interleave
# squares: column split of each sub-block across ACT / DVE / Pool
SQ_ACT = 655
SQ_DVE = 140
# Pool takes the rest (SB - SQ_ACT - SQ_DVE = 238)
MSK_CHUNKS = 2       # mask chain chunks per half tile (one per sub-block)
SQ_ACT_LAST = 660    # last-unit overrides: shift squares toward Pool
SQ_DVE_LAST = 0
SQ_ACT_FIRST = 660   # first-unit overrides
SQ_DVE_FIRST = 140
SB_LAST = 512        # finer sub-blocks for the last unit (tail trim)
SQ_REV_LAST = False  # last unit: Pool range first, ACT last
LOOKAHEAD = 2        # DMA units ahead of compute
IO_BUFS = 3
MK_BUFS = 3
PRED_CHUNKS_FIRST = (678, 346, 1024)  # pred DMA chunks, unit (0,0)
PRED_CHUNKS = (2048,)                 # pred DMA chunks, steady state
PRED_CHUNKS_LAST = (2048,)            # pred DMA chunks, last unit
ITP_WHOLE = False                 # single itp/binp DMA for both images
BINP_WHOLE = False                # single binp DMA for both images
DELTA_V = 0.5
DELTA_D = 3.0

LAST_EXEC_TIME_NS = None

_nc_cache = []


def _build():
    f32, i16, i8, i32 = (mybir.dt.float32, mybir.dt.int16, mybir.dt.int8,
                         mybir.dt.int32)
    fp8 = mybir.dt.float8e4
    op = mybir.AluOpType

    nc = bacc.Bacc("TRN2", target_bir_lowering=False, num_swdge_queues=4)
    pred = nc.dram_tensor("pred", [BPC, 8, 512, 1024], f32,
                          kind="ExternalInput")
    binl = nc.dram_tensor("binl", [BPC, 512, 1024], f32, kind="ExternalInput")
    inst = nc.dram_tensor("inst", [BPC, 512, 1024], i32, kind="ExternalInput")
    out = nc.dram_tensor("out", [BPC, NPL, NCH], f32, kind="ExternalOutput")

    pred_v = pred.rearrange("b e (p a) w -> b p e (a w)", p=P)  # [2,128,8,4096]
    bin_f = binl.rearrange("b (p a) w -> p b (a w)", p=P)       # [128,2,4096]
    inst_f = inst.rearrange("b (p a) w -> p b (a w)", p=P)

    with TileContext(nc) as tc:
        with tc.tile_pool(name="io", bufs=IO_BUFS) as io, \
             tc.tile_pool(name="mk", bufs=MK_BUFS) as mk, \
             tc.tile_pool(name="ii", bufs=1) as ii, \
             tc.tile_pool(name="ps", bufs=2, space="PSUM") as ps, \
             tc.tile_pool(name="res", bufs=2) as res:
            # warm the ACT Square table off the critical path
            az = res.tile([P, 8], fp8, tag="az")
            az2 = res.tile([P, 8], fp8, tag="az2")
            nc.vector.memset(az, 0.0)
            nc.scalar.activation(out=az2, in_=az,
                                 func=mybir.ActivationFunctionType.Square)

            units = [(b, h) for b in range(BPC) for h in range(2)]
            state = {}

            # itp/binp tiles hold BOTH images; DMAs are issued per image
            # inside issue_dma so the first pred chunk transfers early.
            itp = ii.tile([P, 4 * WH], i16, tag="itp")
            binp = ii.tile([P, 4 * WH], i16, tag="binp")

            def issue_dma(unit):
                b, h = unit
                data = io.tile([P, NCH, FH], fp8, tag="data")
                chunks = (PRED_CHUNKS_FIRST if unit == (0, 0) else
                          PRED_CHUNKS_LAST if unit == (BPC - 1, 1) else
                          PRED_CHUNKS)
                it8 = itp.bitcast(i8)
                bi8 = binp.bitcast(fp8)
                if unit == (0, 0):
                    # interleave: itp -> pred c1 -> binp -> pred rest
                    nc.gpsimd.dma_start(out=it8[:, 0:4096],
                                        in_=inst_f[:, 0, :])
                    q0 = chunks[0]
                    nc.gpsimd.dma_start(out=data[:, 0:8, 0:q0],
                                        in_=pred_v[b, :, :, 0:q0])
                    if BINP_WHOLE:
                        nc.gpsimd.dma_start(out=bi8[:, 0:8192],
                                            in_=bin_f[:, :, :])
                    else:
                        nc.gpsimd.dma_start(out=bi8[:, 0:4096],
                                            in_=bin_f[:, 0, :])
                    for cols in chunks[1:]:
                        nc.gpsimd.dma_start(
                            out=data[:, 0:8, q0:q0 + cols],
                            in_=pred_v[b, :, :, q0:q0 + cols])
                        q0 += cols
                else:
                    if h == 0:
                        nc.gpsimd.dma_start(
                            out=it8[:, b * 4096:(b + 1) * 4096],
                            in_=inst_f[:, b, :])
                        if not BINP_WHOLE:
                            nc.gpsimd.dma_start(
                                out=bi8[:, b * 4096:(b + 1) * 4096],
                                in_=bin_f[:, b, :])
                    q0 = 0
                    for cols in chunks:
                        nc.gpsimd.dma_start(
                            out=data[:, 0:8, q0:q0 + cols],
                            in_=pred_v[b, :, :,
                                       h * FH + q0:h * FH + q0 + cols])
                        q0 += cols
                return data

            def compute(unit, data):
                b, h = unit
                u0 = (2 * b + h) * WH   # word offset of this half tile
                msk = mk.tile([P, NPL, WH], i16, tag="msk")
                if h == 0:
                    psum = ps.tile([NPL, NCH], f32, tag="psum")
                    state[b, "ps"] = psum
                psum = state[b, "ps"]
                mv = msk.bitcast(fp8).rearrange(
                    "p m (g i c) -> p g i m c", i=2, c=GRP // 2)
                dv = data.rearrange(
                    "p n (g i c) -> p g i n c", i=2, c=GRP // 2)
                sbsz = SB_LAST if (b, h) == (BPC - 1, 1) else SB
                ngrp_sb = sbsz // GRP
                nsb = FH // sbsz
                last = (b, h) == (BPC - 1, 1)
                first = (b, h) == (0, 0)
                f_act = (SQ_ACT_LAST if last else
                         SQ_ACT_FIRST if first else SQ_ACT) / SB
                f_dve = (SQ_DVE_LAST if last else
                         SQ_DVE_FIRST if first else SQ_DVE) / SB

                # ones slot (int16 view, 4x): (itp*0) + 0x3838
                nc.vector.tensor_scalar(out=data[:, 16, :].bitcast(i16),
                                        in0=itp[:, u0:u0 + WH],
                                        scalar1=0, scalar2=0x3838,
                                        op0=op.mult, op1=op.add)

                for sb in range(nsb):
                    s0 = sb * sbsz            # col offset of sub-block
                    w0, w1 = s0 // 2, (s0 + sbsz) // 2   # word range
                    ith = itp[:, u0 + w0:u0 + w1]
                    bih = binp[:, u0 + w0:u0 + w1]
                    # mask planes for this sub-block
                    for lab in range(NLAB):
                        nc.vector.tensor_scalar(out=msk[:, 5 + lab, w0:w1],
                                                in0=ith,
                                                scalar1=lab * 0x0101,
                                                scalar2=None,
                                                op0=op.bitwise_xor)
                    nc.vector.tensor_scalar(out=msk[:, 5:10, w0:w1],
                                            in0=msk[:, 5:10, w0:w1],
                                            scalar1=0x3F3F, scalar2=None,
                                            op0=op.add)
                    nc.vector.tensor_scalar(out=msk[:, 5:10, w0:w1],
                                            in0=msk[:, 5:10, w0:w1],
                                            scalar1=0x4040, scalar2=0x4040,
                                            op0=op.bitwise_and,
                                            op1=op.bitwise_xor)
                    nc.vector.tensor_tensor(
                        out=msk[:, 0:5, w0:w1], in0=msk[:, 5:10, w0:w1],
                        in1=bih[:, None, :].broadcast_to([P, NLAB, w1 - w0]),
                        op=op.bitwise_and)

                    # squares for this sub-block
                    na = int(f_act * sbsz)
                    nd = int(f_dve * sbsz)
                    c0, c1 = s0 + na, s0 + na + nd
                    if last and SQ_REV_LAST:
                        na = c0 - s0
                        nd = c1 - c0
                        npl = sbsz - na - nd
                        r0, r1 = s0 + npl, s0 + npl + nd
                        if npl:
                            nc.gpsimd.tensor_tensor(
                                out=data[:, 8:16, s0:r0],
                                in0=data[:, 0:8, s0:r0],
                                in1=data[:, 0:8, s0:r0], op=op.mult)
                        if nd:
                            nc.vector.scalar_tensor_tensor(
                                out=data[:, 8:16, r0:r1],
                                in0=data[:, 0:8, r0:r1],
                                scalar=1.0, in1=data[:, 0:8, r0:r1],
                                op0=op.mult, op1=op.mult)
                        if na:
                            nc.scalar.activation(
                                out=data[:, 8:16, r1:s0 + sbsz],
                                in_=data[:, 0:8, r1:s0 + sbsz],
                                func=mybir.ActivationFunctionType.Square)
                    else:
                        if c0 > s0:
                            nc.scalar.activation(
                                out=data[:, 8:16, s0:c0],
                                in_=data[:, 0:8, s0:c0],
                                func=mybir.ActivationFunctionType.Square)
                        if c1 > c0:
                            nc.vector.scalar_tensor_tensor(
                                out=data[:, 8:16, c0:c1],
                                in0=data[:, 0:8, c0:c1],
                                scalar=1.0, in1=data[:, 0:8, c0:c1],
                                op0=op.mult, op1=op.mult)
                        if s0 + sbsz > c1:
                            nc.gpsimd.tensor_tensor(
                                out=data[:, 8:16, c1:s0 + sbsz],
                                in0=data[:, 0:8, c1:s0 + sbsz],
                                in1=data[:, 0:8, c1:s0 + sbsz], op=op.mult)

                    # matmuls for this sub-block
                    for g in range(sb * ngrp_sb, (sb + 1) * ngrp_sb):
                        for k in range(GRP // 2):
                            first = (h == 0 and g == 0 and k == 0)
                            last = (h == 1 and g == FH // GRP - 1
                                    and k == GRP // 2 - 1)
                            nc.tensor.matmul(psum[:, :], mv[:, g, :, :, k],
                                             dv[:, g, :, :, k],
                                             start=first, stop=last,
                                             perf_mode=mybir.MatmulPerfMode.
                                             DoubleRow)

                if h == 1:
                    ot = res.tile([P, NCH], f32, tag="ot")
                    nc.vector.tensor_copy(out=ot[0:NPL, :], in_=psum[:, :])
                    nc.sync.dma_start(out=out[b, :, :], in_=ot[0:NPL, :])

            bufs = {}
            for u in range(min(LOOKAHEAD, len(units))):
                bufs[u] = issue_dma(units[u])
            for u, unit in enumerate(units):
                la = u + LOOKAHEAD
                if la < len(units):
                    bufs[la] = issue_dma(units[la])
                compute(unit, bufs.pop(u))
    nc.compile()
    return nc


def _get_nc():
    if not _nc_cache:
        _nc_cache.append(_build())
    return _nc_cache[0]


def _loss_from_stats(sums, sumsq, counts):
    """Mirror of the reference loss math in float64.
    sums [B,5,8], sumsq [B,5], counts [B,5]."""
    C = NLAB - 1
    with np.errstate(divide="ignore", invalid="ignore"):
        mu = sums / counts[..., None]                         # [B,5,8]
    frob = sumsq - counts * np.sum(mu * mu, axis=-1)          # [B,5]
    pos = frob > 0
    n = np.where(pos, np.sqrt(np.where(pos, frob, 1.0)), 0.0)
    var = np.where(n > DELTA_V, (n - DELTA_V) ** 2, 0.0)
    l_var = np.sum(var, axis=1) / C                           # [B]

    mu_d = mu[:, :C]                                          # [B,4,8]
    diff = mu_d[:, :, None, :] - mu_d[:, None, :, :]
    dsq = np.sum(diff * diff, axis=-1)                        # [B,4,4]
    offdiag = (1.0 - np.eye(C))[None]
    ok = (dsq > 0) & (offdiag > 0)
    d = np.sqrt(np.where(ok, dsq, 1.0))
    hinge = np.where(ok, np.maximum(DELTA_D - d, 0.0) ** 2,
                     np.where(offdiag > 0, DELTA_D ** 2, 0.0))
    l_dist = np.sum(hinge, axis=(1, 2))                       # [B]
    return np.mean(l_var) + np.mean(l_dist)


def kernel(pred, binary_label, instance_label):
    global LAST_EXEC_TIME_NS
    pred = np.ascontiguousarray(pred, dtype=np.float32)
    # *2 so the f32->fp8 DMA cast yields bit pattern 0x40, matching the
    # 0x40-coded masks for the bitwise AND.
    binl = np.ascontiguousarray(
        binary_label, dtype=np.float32).reshape(pred.shape[0], 512, 1024) * 2.0
    inst = np.ascontiguousarray(instance_label, dtype=np.int32)

    nc = _get_nc()
    in_maps = []
    for c in range(NCORES):
        sl = slice(BPC * c, BPC * (c + 1))
        in_maps.append({
            "pred": np.ascontiguousarray(pred[sl]),
            "binl": np.ascontiguousarray(binl[sl]),
            "inst": np.ascontiguousarray(inst[sl]),
        })

    r = bass_utils.run_bass_kernel_spmd(nc, in_maps,
                                        core_ids=list(range(NCORES)))
    LAST_EXEC_TIME_NS = r.exec_time_ns

    packed = np.stack([m["out"] for m in r.results]).reshape(
        NCORES * BPC, NPL, NCH).astype(np.float64)
    sums = packed[:, 0:5, 0:8] / 2.0
    sumsq = packed[:, 0:5, 8:16].sum(-1) / 2.0
    counts = packed[:, 5:10, 16] / 2.0

    loss = _loss_from_stats(sums, sumsq, counts)
    return np.array(loss, dtype=np.float32)
